# revision 13
# baseline (speedup 1.0000x reference)
"""DNDT forward kernel for Trainium2 (8 NeuronCores, data-parallel).

Math (matches the reference):
    w = [1,2,3,4];  b = [0, cumsum(-sort(beta))]
    sigma[i,f,k] = sigmoid((x[i,f]*w[k] + b[k]) / T)            [B, 6, 4]
    leaves[i]    = kron(sigma[i,0], ..., sigma[i,5])            [B, 4096]
    out          = leaves @ L                                   [B, 10]

Restructured to avoid materializing the 4096-wide leaves:
    A[i,a]  = kron(s0, s1)          a = k0*4+k1      in [0,16)
    Bm[i,b] = kron(s2, s3, s4, s5)  b in [0,256)
    M[i,(c,a)] = sum_b Bm[i,b] * L3[b, (c,a)]   (PE matmul, K=256 in 2 chunks)
    out[i,c]   = sum_a A[i,a] * M[i,(c,a)]      (DVE multiply + pairwise adds)

Device tricks:
  - z = (x*w+b)/T precomputed on host, shipped f16 in supertile layout.
  - Bm columns ordered so consecutive f16 pairs belong to the two K-chunks
    (uu-parity).  One PE transpose of the f32-bitcast [128,128] tile yields
    both chunks' weights; LDWEIGHTS reads them with a stride-2 f16 AP.
    The compensating L3 row permutation is done on host.
  - M is (c,a)-major (host permutes L3 columns) so the A-contraction is a
    3-free-dim fp16 tensor_tensor at DVE 2x (stride-1 innermost for both
    operands, no duplication), followed by pairwise adds.
  - Cheap stages (sigmoid, krons, A, prod, add-tree) are fused across
    supertile blocks [0],[1,2],[3,4],[5,6],[7] to amortize per-op engine
    overhead; bm + transpose + matmul stay per-supertile for pipelining.
  - Supertile 0's krons run on the (otherwise idle at ramp) DVE; later
    blocks' krons run on GpSimd one block ahead of use.
  - PSUM->SBUF evacuation of M is split: m-group 0 on Scalar, 1 on GpSimd.
  - prod/tree/output for block b are emitted after block b+1's matmuls
    (software pipelining), so the DVE never waits on the PE/Scalar chain.

Per-core: 8192 rows as 8 supertiles of 1024 rows; partition p holds rows
{st*1024 + p*8 + g : g in [0,8)}.
"""

import numpy as np

import concourse.bacc as bacc
import concourse.mybir as mybir
import concourse.tile as tile
from concourse.bass_utils import run_bass_kernel_spmd

F32 = mybir.dt.float32
F16 = mybir.dt.float16

B, F, NB, NCLS = 65536, 6, 4, 10
CORES = 8
ROWS = B // CORES          # 8192 rows per core
G = 8                      # row-groups per supertile
ST_ROWS = 128 * G          # 1024 rows per supertile
N_ST = ROWS // ST_ROWS     # 8 supertiles
TEMP = 0.1

BLOCKS = [[0], [1, 2], [3, 4], [5, 6], [7]]

_NC_CACHE = {}


def _build_nc():
    nc = bacc.Bacc("TRN2", target_bir_lowering=False, debug=False)

    zt = nc.dram_tensor("zt", [128, N_ST, G, 24], F16, kind="ExternalInput")
    l3p = nc.dram_tensor("l3p", [128, 2, 160], F16, kind="ExternalInput")
    identf = nc.dram_tensor("identf", [128, 128], F32, kind="ExternalInput")
    outc = nc.dram_tensor("outc", [128, N_ST, G, NCLS], F32, kind="ExternalOutput")

    with tile.TileContext(nc) as tc:
        with (
            tc.tile_pool(name="big", bufs=1) as big,
            tc.tile_pool(name="work", bufs=3) as work,
            tc.tile_pool(name="io", bufs=2) as io,
            tc.tile_pool(name="wts", bufs=3) as wts,
            tc.tile_pool(name="ps_t", bufs=2, space="PSUM") as ps_t,
            tc.tile_pool(name="ps_m", bufs=2, space="PSUM") as ps_m,
        ):
            # singly-written resident tensors
            z_sb = big.tile([128, N_ST, G, 24], F16)
            sig = big.tile([128, N_ST, G, 24], F16)
            u_t = big.tile([128, N_ST, G, 16], F16)
            s5p = big.tile([128, N_ST, G, NB, 2], F16)
            vp2 = big.tile([128, N_ST, G, 16, 2], F16)
            a_t = big.tile([128, N_ST, G, 16], F16)
            msb = big.tile([128, N_ST, G, 160], F16)
            l3_sb = big.tile([128, 2, 160], F16)
            id_sb = big.tile([128, 128], F32)

            # input DMAs: z for the first three supertiles, then consts
            # (needed ~2us later), then the rest of z
            nc.sync.dma_start(z_sb[:, 0:1], zt[:, 0:1])
            nc.sync.dma_start(z_sb[:, 1:3], zt[:, 1:3])
            nc.sync.dma_start(id_sb[:, :], identf[:, :])
            nc.sync.dma_start(l3_sb[:, :, :], l3p[:, :, :])
            nc.sync.dma_start(z_sb[:, 3:5], zt[:, 3:5])
            nc.sync.dma_start(z_sb[:, 5:7], zt[:, 5:7])
            nc.sync.dma_start(z_sb[:, 7:8], zt[:, 7:8])

            def blk_view(t, blk, tail):
                # [128, n*G] + tail view of a big tensor's block slice
                n = len(blk)
                return t[:, blk[0]:blk[0] + n].rearrange(
                    f"p n g {tail} -> p (n g) {tail}")

            def sigmoid(blk):
                k0, n = blk[0], len(blk)
                nc.scalar.activation(
                    sig[:, k0:k0 + n], z_sb[:, k0:k0 + n],
                    mybir.ActivationFunctionType.Sigmoid,
                )

            def krons(blk, eng):
                # u = s2 x s3;  vp2[.., vv, t] = s4[k4]*s5[k5] (dup via
                # stride-0 reads, no separate dup op)
                ng = len(blk) * G
                sg = blk_view(sig, blk, "c")            # [128, ng, 24]
                eng.tensor_mul(
                    blk_view(u_t, blk, "a").rearrange(
                        "p s (i j) -> p s i j", j=NB),
                    sg[:, :, 8:12].unsqueeze(3).broadcast_to((128, ng, NB, NB)),
                    sg[:, :, 12:16].unsqueeze(2).broadcast_to((128, ng, NB, NB)),
                )
                nc.vector.tensor_copy(
                    blk_view(s5p, blk, "j t"),
                    sg[:, :, 20:24].unsqueeze(3).broadcast_to((128, ng, NB, 2)),
                )
                eng.tensor_mul(
                    blk_view(vp2, blk, "v t").rearrange(
                        "p s (i j) t -> p s i (j t)", j=NB),
                    sg[:, :, 16:20].unsqueeze(3).broadcast_to(
                        (128, ng, NB, 2 * NB)),
                    blk_view(s5p, blk, "j t").rearrange(
                        "p s j t -> p s (j t)").unsqueeze(2).broadcast_to(
                        (128, ng, NB, 2 * NB)),
                )

            def a_kron(blk, eng):
                ng = len(blk) * G
                sg = blk_view(sig, blk, "c")
                eng.tensor_mul(
                    blk_view(a_t, blk, "a").rearrange(
                        "p s (i j) -> p s i j", j=NB),
                    sg[:, :, 0:4].unsqueeze(3).broadcast_to((128, ng, NB, NB)),
                    sg[:, :, 4:8].unsqueeze(2).broadcast_to((128, ng, NB, NB)),
                )

            def stage_C(blk):
                # prod + pairwise-add tree + output DMA for a block
                k0, n = blk[0], len(blk)
                ng = n * G
                prodc = work.tile([128, ng, NCLS, 16], F16, tag="prodc")
                nc.vector.tensor_mul(
                    prodc[:, :, :, :],
                    blk_view(a_t, blk, "a").unsqueeze(2).broadcast_to(
                        (128, ng, NCLS, 16)),
                    blk_view(msb, blk, "ca").rearrange(
                        "p s (c a) -> p s c a", a=16),
                )
                f1 = work.tile([128, ng, NCLS, 8], F16, tag="f1")
                nc.vector.tensor_add(
                    f1[:, :, :, :], prodc[:, :, :, 0:8], prodc[:, :, :, 8:16])
                f2 = work.tile([128, ng, NCLS, 4], F16, tag="f2")
                nc.vector.tensor_add(
                    f2[:, :, :, :], f1[:, :, :, 0:4], f1[:, :, :, 4:8])
                f3 = work.tile([128, ng, NCLS, 2], F16, tag="f3")
                nc.vector.tensor_add(
                    f3[:, :, :, :], f2[:, :, :, 0:2], f2[:, :, :, 2:4])
                oq = io.tile([128, ng, NCLS], F32, tag="oq")
                nc.vector.tensor_add(oq[:, :, :], f3[:, :, :, 0], f3[:, :, :, 1])
                nc.sync.dma_start(
                    outc[:, k0:k0 + n],
                    oq[:, :, :].rearrange("p (n g) c -> p n g c", n=n))

            def stage_B(k):
                # transposes + matmuls + M evacuation for supertile k
                bm = state[k]
                tp = ps_t.tile([128, G, 128], F32, tag="tp")
                for q in range(G):
                    nc.tensor.transpose(
                        tp[:, q, :], bm[:, q, :].bitcast(F32), id_sb[:, :])
                bmt = wts.tile([128, G, 128], F32, tag="bmt")
                nc.scalar.copy(bmt[:, :, :], tp[:, :, :])
                for m in range(2):
                    mps = ps_m.tile([128, 4, 256], F32, tag="m")
                    for qq in range(4):
                        q = m * 4 + qq
                        w16 = bmt[:, q, :].bitcast(F16).rearrange(
                            "p (j t) -> p t j", t=2)
                        nc.tensor.matmul(
                            mps[:, qq, 0:160], w16[:, 0, :], l3_sb[:, 0, :],
                            start=True, stop=False,
                        )
                        nc.tensor.matmul(
                            mps[:, qq, 0:160], w16[:, 1, :], l3_sb[:, 1, :],
                            start=False, stop=True,
                        )
                    nc.scalar.copy(
                        msb[:, k, m * 4:(m + 1) * 4, :], mps[:, :, 0:160])

            state = {}

            # sigmoids for every block up front (scalar)
            for blk in BLOCKS:
                sigmoid(blk)
            # block 0 krons on the ramp-idle DVE; block 1 on gpsimd
            krons(BLOCKS[0], nc.vector)
            krons(BLOCKS[1], nc.gpsimd)

            GP_BM = (5, 6, 7)   # late supertiles' bm built by gpsimd

            for bi, blk in enumerate(BLOCKS):
                if bi + 2 < len(BLOCKS):
                    krons(BLOCKS[bi + 2], nc.gpsimd)
                a_kron(blk, nc.gpsimd)
                for k in blk:
                    # Bm[p,g, vv*16 + uu] = u[p,g,uu] * v[p,g,vv]
                    bm = work.tile([128, G, 256], F16, tag="bm")
                    if k in GP_BM:
                        nc.gpsimd.tensor_mul(
                            bm[:, :, :].rearrange("p g (i u) -> p g i u", u=16),
                            u_t[:, k].unsqueeze(2).broadcast_to(
                                (128, G, 16, 16)),
                            vp2[:, k, :, :, 0].unsqueeze(3).broadcast_to(
                                (128, G, 16, 16)),
                        )
                    else:
                        nc.vector.tensor_mul(
                            bm[:, :, :].rearrange(
                                "p g (i j t) -> p g i j t", j=8, t=2),
                            u_t[:, k].rearrange("p g (j t) -> p g j t", t=2)
                                .unsqueeze(2).broadcast_to((128, G, 16, 8, 2)),
                            vp2[:, k].unsqueeze(3).broadcast_to(
                                (128, G, 16, 8, 2)),
                        )
                    state[k] = bm
                    stage_B(k)
                if bi > 0:
                    stage_C(BLOCKS[bi - 1])
            stage_C(BLOCKS[-1])

    nc.compile()
    return nc


def _host_prep(x, beta, leaves2classes):
    x = np.asarray(x, dtype=np.float32)
    beta = np.asarray(beta, dtype=np.float32)
    L = np.asarray(leaves2classes, dtype=np.float32)

    w = np.linspace(1.0, float(NB), NB, dtype=np.float32)
    bs = np.sort(beta)
    b = np.concatenate([np.zeros(1, np.float32), np.cumsum(-bs, dtype=np.float32)])

    # z[i, f*4+k] = (x[i,f]*w[k] + b[k]) / T
    z = (x[:, :, None] * w[None, None, :] + b[None, None, :]) / np.float32(TEMP)
    z = z.reshape(B, F * NB).astype(np.float16)

    # L3r[b_leaf, a, c] = L[a*256 + b_leaf, c]
    L3r = L.reshape(16, 256, NCLS).transpose(1, 0, 2)      # [256, 16, 10]
    # device Bm column order: col = vv*16 + uu, packed pair (2J, 2J+1):
    #   J = vv*8 + uu//2, T = uu%2;  b_leaf = uu*16 + vv
    J = np.arange(128)
    l3p = np.empty((128, 2, 160), np.float32)
    for T in range(2):
        uu = 2 * (J % 8) + T
        vv = J // 8
        rows = L3r[uu * 16 + vv]                           # [128, 16, 10]
        # columns (c, a)-major: col = c*16 + a
        l3p[:, T, :] = rows.transpose(0, 2, 1).reshape(128, 160)
    l3p = l3p.astype(np.float16)

    ident = np.eye(128, dtype=np.float32)
    return z, l3p, ident


def _prep_in_maps(x, beta, leaves2classes):
    z, l3p, ident = _host_prep(x, beta, leaves2classes)
    in_maps = []
    for c in range(CORES):
        zc = z[c * ROWS:(c + 1) * ROWS].reshape(N_ST, 128, G, 24)
        zc = np.ascontiguousarray(zc.transpose(1, 0, 2, 3))
        in_maps.append({"zt": zc, "l3p": l3p, "identf": ident})
    return in_maps


def kernel(x, beta, leaves2classes):
    in_maps = _prep_in_maps(x, beta, leaves2classes)

    if "nc" not in _NC_CACHE:
        _NC_CACHE["nc"] = _build_nc()
    nc = _NC_CACHE["nc"]

    res = run_bass_kernel_spmd(nc, in_maps, core_ids=list(range(CORES)))
    outs = []
    for r in res.results:
        oc = r["outc"]                                     # [128, N_ST, G, 10]
        outs.append(oc.transpose(1, 0, 2, 3).reshape(ROWS, NCLS))
    return np.concatenate(outs, axis=0).astype(np.float32)


# revision 14
# speedup vs baseline: 1.1840x; 1.1840x over previous
"""DNDT forward kernel for Trainium2 (8 NeuronCores, data-parallel).

Math (matches the reference):
    w = [1,2,3,4];  b = [0, cumsum(-sort(beta))]
    sigma[i,f,k] = sigmoid((x[i,f]*w[k] + b[k]) / T)            [B, 6, 4]
    leaves[i]    = kron(sigma[i,0], ..., sigma[i,5])            [B, 4096]
    out          = leaves @ L                                   [B, 10]

Restructured to avoid materializing the 4096-wide leaves:
    A[i,a]  = kron(s0, s1)          a = k0*4+k1      in [0,16)
    Bm[i,b] = kron(s2, s3, s4, s5)  b in [0,256)
    M[i,(c,a)] = sum_b Bm[i,b] * L3[b, (c,a)]   (PE matmul, K=256 in 2 chunks)
    out[i,c]   = sum_a A[i,a] * M[i,(c,a)]      (DVE multiply + pairwise adds)

Device tricks:
  - z = (x*w+b)/T precomputed on host, shipped f16 in supertile layout.
  - Bm columns ordered so consecutive f16 pairs belong to the two K-chunks
    (uu-parity).  One PE transpose of the f32-bitcast [128,128] tile yields
    both chunks' weights; LDWEIGHTS reads them with a stride-2 f16 AP.
    The compensating L3 row permutation is done on host.
  - M is (c,a)-major (host permutes L3 columns) so the A-contraction is a
    3-free-dim fp16 tensor_tensor at DVE 2x (stride-1 innermost for both
    operands, no duplication), followed by pairwise adds.
  - Cheap stages (sigmoid, krons, A, prod, add-tree) are fused across
    supertile blocks [0],[1,2],[3,4],[5,6],[7] to amortize per-op engine
    overhead; bm + transpose + matmul stay per-supertile for pipelining.
  - Supertile 0's krons run on the (otherwise idle at ramp) DVE; later
    blocks' krons run on GpSimd one block ahead of use.
  - PSUM->SBUF evacuation of M is split: m-group 0 on Scalar, 1 on GpSimd.
  - prod/tree/output for block b are emitted after block b+1's matmuls
    (software pipelining), so the DVE never waits on the PE/Scalar chain.

Per-core: 8192 rows as 8 supertiles of 1024 rows; partition p holds rows
{st*1024 + p*8 + g : g in [0,8)}.
"""

import numpy as np

import concourse.bacc as bacc
import concourse.mybir as mybir
import concourse.tile as tile
from concourse.bass_utils import run_bass_kernel_spmd

F32 = mybir.dt.float32
F16 = mybir.dt.float16

B, F, NB, NCLS = 65536, 6, 4, 10
CORES = 8
ROWS = B // CORES          # 8192 rows per core
G = 8                      # row-groups per supertile
ST_ROWS = 128 * G          # 1024 rows per supertile
N_ST = ROWS // ST_ROWS     # 8 supertiles
TEMP = 0.1

BLOCKS = [[0], [1, 2], [3, 4], [5, 6], [7]]

_NC_CACHE = {}


def _build_nc():
    nc = bacc.Bacc("TRN2", target_bir_lowering=False, debug=False)

    zt = nc.dram_tensor("zt", [128, N_ST, G, 24], F16, kind="ExternalInput")
    l3p = nc.dram_tensor("l3p", [128, 2, 160], F16, kind="ExternalInput")
    identf = nc.dram_tensor("identf", [128, 128], F32, kind="ExternalInput")
    outc = nc.dram_tensor("outc", [128, N_ST, G, NCLS], F32, kind="ExternalOutput")

    with tile.TileContext(nc) as tc:
        with (
            tc.tile_pool(name="big", bufs=1) as big,
            tc.tile_pool(name="work", bufs=3) as work,
            tc.tile_pool(name="io", bufs=2) as io,
            tc.tile_pool(name="wts", bufs=3) as wts,
            tc.tile_pool(name="ps_t", bufs=2, space="PSUM") as ps_t,
            tc.tile_pool(name="ps_m", bufs=2, space="PSUM") as ps_m,
        ):
            # singly-written resident tensors
            z_sb = big.tile([128, N_ST, G, 24], F16)
            sig = big.tile([128, N_ST, G, 24], F16)
            u_t = big.tile([128, N_ST, G, 16], F16)
            s5p = big.tile([128, N_ST, G, NB, 2], F16)
            vp2 = big.tile([128, N_ST, G, 16, 2], F16)
            a_t = big.tile([128, N_ST, G, 16], F16)
            msb = big.tile([128, N_ST, G, 160], F16)
            l3_sb = big.tile([128, 2, 160], F16)
            id_sb = big.tile([128, 128], F32)

            # input DMAs: z for the first three supertiles, then consts
            # (needed ~2us later), then the rest of z
            nc.sync.dma_start(z_sb[:, 0:1], zt[:, 0:1])
            nc.sync.dma_start(z_sb[:, 1:3], zt[:, 1:3])
            nc.sync.dma_start(id_sb[:, :], identf[:, :])
            nc.sync.dma_start(l3_sb[:, :, :], l3p[:, :, :])
            nc.sync.dma_start(z_sb[:, 3:5], zt[:, 3:5])
            nc.sync.dma_start(z_sb[:, 5:7], zt[:, 5:7])
            nc.sync.dma_start(z_sb[:, 7:8], zt[:, 7:8])

            def blk_view(t, blk, tail):
                # [128, n*G] + tail view of a big tensor's block slice
                n = len(blk)
                return t[:, blk[0]:blk[0] + n].rearrange(
                    f"p n g {tail} -> p (n g) {tail}")

            def sigmoid(blk):
                k0, n = blk[0], len(blk)
                nc.scalar.activation(
                    sig[:, k0:k0 + n], z_sb[:, k0:k0 + n],
                    mybir.ActivationFunctionType.Sigmoid,
                )

            def krons(blk, eng):
                # u = s2 x s3;  vp2[.., vv, t] = s4[k4]*s5[k5] (dup via
                # stride-0 reads, no separate dup op)
                ng = len(blk) * G
                sg = blk_view(sig, blk, "c")            # [128, ng, 24]
                eng.tensor_mul(
                    blk_view(u_t, blk, "a").rearrange(
                        "p s (i j) -> p s i j", j=NB),
                    sg[:, :, 8:12].unsqueeze(3).broadcast_to((128, ng, NB, NB)),
                    sg[:, :, 12:16].unsqueeze(2).broadcast_to((128, ng, NB, NB)),
                )
                nc.vector.tensor_copy(
                    blk_view(s5p, blk, "j t"),
                    sg[:, :, 20:24].unsqueeze(3).broadcast_to((128, ng, NB, 2)),
                )
                eng.tensor_mul(
                    blk_view(vp2, blk, "v t").rearrange(
                        "p s (i j) t -> p s i (j t)", j=NB),
                    sg[:, :, 16:20].unsqueeze(3).broadcast_to(
                        (128, ng, NB, 2 * NB)),
                    blk_view(s5p, blk, "j t").rearrange(
                        "p s j t -> p s (j t)").unsqueeze(2).broadcast_to(
                        (128, ng, NB, 2 * NB)),
                )

            def a_kron(blk, eng):
                ng = len(blk) * G
                sg = blk_view(sig, blk, "c")
                eng.tensor_mul(
                    blk_view(a_t, blk, "a").rearrange(
                        "p s (i j) -> p s i j", j=NB),
                    sg[:, :, 0:4].unsqueeze(3).broadcast_to((128, ng, NB, NB)),
                    sg[:, :, 4:8].unsqueeze(2).broadcast_to((128, ng, NB, NB)),
                )

            def stage_C(blk):
                # prod + pairwise-add tree + output DMA for a block
                k0, n = blk[0], len(blk)
                ng = n * G
                prodc = work.tile([128, ng, NCLS, 16], F16, tag="prodc")
                nc.vector.tensor_mul(
                    prodc[:, :, :, :],
                    blk_view(a_t, blk, "a").unsqueeze(2).broadcast_to(
                        (128, ng, NCLS, 16)),
                    blk_view(msb, blk, "ca").rearrange(
                        "p s (c a) -> p s c a", a=16),
                )
                f1 = work.tile([128, ng, NCLS, 8], F16, tag="f1")
                nc.vector.tensor_add(
                    f1[:, :, :, :], prodc[:, :, :, 0:8], prodc[:, :, :, 8:16])
                f2 = work.tile([128, ng, NCLS, 4], F16, tag="f2")
                nc.vector.tensor_add(
                    f2[:, :, :, :], f1[:, :, :, 0:4], f1[:, :, :, 4:8])
                f3 = work.tile([128, ng, NCLS, 2], F16, tag="f3")
                nc.vector.tensor_add(
                    f3[:, :, :, :], f2[:, :, :, 0:2], f2[:, :, :, 2:4])
                oq = io.tile([128, ng, NCLS], F32, tag="oq")
                nc.vector.tensor_add(oq[:, :, :], f3[:, :, :, 0], f3[:, :, :, 1])
                nc.sync.dma_start(
                    outc[:, k0:k0 + n],
                    oq[:, :, :].rearrange("p (n g) c -> p n g c", n=n))

            def stage_B(k):
                # transposes + matmuls + M evacuation for supertile k
                bm = state[k]
                tp = ps_t.tile([128, G, 128], F32, tag="tp")
                for q in range(G):
                    nc.tensor.transpose(
                        tp[:, q, :], bm[:, q, :].bitcast(F32), id_sb[:, :])
                bmt = wts.tile([128, G, 128], F32, tag="bmt")
                nc.scalar.copy(bmt[:, :, :], tp[:, :, :])
                for m in range(2):
                    mps = ps_m.tile([128, 4, 256], F32, tag="m")
                    for qq in range(4):
                        q = m * 4 + qq
                        w16 = bmt[:, q, :].bitcast(F16).rearrange(
                            "p (j t) -> p t j", t=2)
                        nc.tensor.matmul(
                            mps[:, qq, 0:160], w16[:, 0, :], l3_sb[:, 0, :],
                            start=True, stop=False,
                        )
                        nc.tensor.matmul(
                            mps[:, qq, 0:160], w16[:, 1, :], l3_sb[:, 1, :],
                            start=False, stop=True,
                        )
                    nc.scalar.copy(
                        msb[:, k, m * 4:(m + 1) * 4, :], mps[:, :, 0:160])

            state = {}

            # sigmoids for every block up front (scalar)
            for blk in BLOCKS:
                sigmoid(blk)
            # block 0 krons on the ramp-idle DVE; block 1 on gpsimd
            krons(BLOCKS[0], nc.vector)
            krons(BLOCKS[1], nc.gpsimd)

            GP_BM = ()          # gpsimd bm build measured ~4us/supertile: off

            for bi, blk in enumerate(BLOCKS):
                if bi + 2 < len(BLOCKS):
                    krons(BLOCKS[bi + 2], nc.gpsimd)
                a_kron(blk, nc.gpsimd)
                for k in blk:
                    # Bm[p,g, vv*16 + uu] = u[p,g,uu] * v[p,g,vv]
                    bm = work.tile([128, G, 256], F16, tag="bm")
                    if k in GP_BM:
                        nc.gpsimd.tensor_mul(
                            bm[:, :, :].rearrange("p g (i u) -> p g i u", u=16),
                            u_t[:, k].unsqueeze(2).broadcast_to(
                                (128, G, 16, 16)),
                            vp2[:, k, :, :, 0].unsqueeze(3).broadcast_to(
                                (128, G, 16, 16)),
                        )
                    else:
                        nc.vector.tensor_mul(
                            bm[:, :, :].rearrange(
                                "p g (i j t) -> p g i j t", j=8, t=2),
                            u_t[:, k].rearrange("p g (j t) -> p g j t", t=2)
                                .unsqueeze(2).broadcast_to((128, G, 16, 8, 2)),
                            vp2[:, k].unsqueeze(3).broadcast_to(
                                (128, G, 16, 8, 2)),
                        )
                    state[k] = bm
                    stage_B(k)
                if bi > 0:
                    stage_C(BLOCKS[bi - 1])
            stage_C(BLOCKS[-1])

    nc.compile()
    return nc


def _host_prep(x, beta, leaves2classes):
    x = np.asarray(x, dtype=np.float32)
    beta = np.asarray(beta, dtype=np.float32)
    L = np.asarray(leaves2classes, dtype=np.float32)

    w = np.linspace(1.0, float(NB), NB, dtype=np.float32)
    bs = np.sort(beta)
    b = np.concatenate([np.zeros(1, np.float32), np.cumsum(-bs, dtype=np.float32)])

    # z[i, f*4+k] = (x[i,f]*w[k] + b[k]) / T
    z = (x[:, :, None] * w[None, None, :] + b[None, None, :]) / np.float32(TEMP)
    z = z.reshape(B, F * NB).astype(np.float16)

    # L3r[b_leaf, a, c] = L[a*256 + b_leaf, c]
    L3r = L.reshape(16, 256, NCLS).transpose(1, 0, 2)      # [256, 16, 10]
    # device Bm column order: col = vv*16 + uu, packed pair (2J, 2J+1):
    #   J = vv*8 + uu//2, T = uu%2;  b_leaf = uu*16 + vv
    J = np.arange(128)
    l3p = np.empty((128, 2, 160), np.float32)
    for T in range(2):
        uu = 2 * (J % 8) + T
        vv = J // 8
        rows = L3r[uu * 16 + vv]                           # [128, 16, 10]
        # columns (c, a)-major: col = c*16 + a
        l3p[:, T, :] = rows.transpose(0, 2, 1).reshape(128, 160)
    l3p = l3p.astype(np.float16)

    ident = np.eye(128, dtype=np.float32)
    return z, l3p, ident


def _prep_in_maps(x, beta, leaves2classes):
    z, l3p, ident = _host_prep(x, beta, leaves2classes)
    in_maps = []
    for c in range(CORES):
        zc = z[c * ROWS:(c + 1) * ROWS].reshape(N_ST, 128, G, 24)
        zc = np.ascontiguousarray(zc.transpose(1, 0, 2, 3))
        in_maps.append({"zt": zc, "l3p": l3p, "identf": ident})
    return in_maps


def kernel(x, beta, leaves2classes):
    in_maps = _prep_in_maps(x, beta, leaves2classes)

    if "nc" not in _NC_CACHE:
        _NC_CACHE["nc"] = _build_nc()
    nc = _NC_CACHE["nc"]

    res = run_bass_kernel_spmd(nc, in_maps, core_ids=list(range(CORES)))
    outs = []
    for r in res.results:
        oc = r["outc"]                                     # [128, N_ST, G, 10]
        outs.append(oc.transpose(1, 0, 2, 3).reshape(ROWS, NCLS))
    return np.concatenate(outs, axis=0).astype(np.float32)


# revision 16
# speedup vs baseline: 1.2097x; 1.0218x over previous
"""DNDT forward kernel for Trainium2 (8 NeuronCores, data-parallel).

Math (matches the reference):
    w = [1,2,3,4];  b = [0, cumsum(-sort(beta))]
    sigma[i,f,k] = sigmoid((x[i,f]*w[k] + b[k]) / T)            [B, 6, 4]
    leaves[i]    = kron(sigma[i,0], ..., sigma[i,5])            [B, 4096]
    out          = leaves @ L                                   [B, 10]

Restructured to avoid materializing the 4096-wide leaves:
    A[i,a]  = kron(s0, s1)          a = k0*4+k1      in [0,16)
    Bm[i,b] = kron(s2, s3, s4, s5)  b in [0,256)
    M[i,(c,a)] = sum_b Bm[i,b] * L3[b, (c,a)]   (PE matmul, K=256 in 2 chunks)
    out[i,c]   = sum_a A[i,a] * M[i,(c,a)]      (DVE multiply + pairwise adds)

Device tricks:
  - z = (x*w+b)/T precomputed on host, shipped f16 in supertile layout.
  - Bm columns ordered so consecutive f16 pairs belong to the two K-chunks
    (uu-parity).  One PE transpose of the f32-bitcast [128,128] tile yields
    both chunks' weights; LDWEIGHTS reads them with a stride-2 f16 AP.
    The compensating L3 row permutation is done on host.
  - M is (c,a)-major (host permutes L3 columns) so the A-contraction is a
    3-free-dim fp16 tensor_tensor at DVE 2x (stride-1 innermost for both
    operands, no duplication), followed by pairwise adds.
  - Cheap stages (sigmoid, krons, A, prod, add-tree) are fused across
    supertile blocks [0],[1,2],[3,4],[5,6],[7] to amortize per-op engine
    overhead; bm + transpose + matmul stay per-supertile for pipelining.
  - Supertile 0's krons run on the (otherwise idle at ramp) DVE; later
    blocks' krons run on GpSimd one block ahead of use.
  - PSUM->SBUF evacuation of M is split: m-group 0 on Scalar, 1 on GpSimd.
  - prod/tree/output for block b are emitted after block b+1's matmuls
    (software pipelining), so the DVE never waits on the PE/Scalar chain.

Per-core: 8192 rows as 8 supertiles of 1024 rows; partition p holds rows
{st*1024 + p*8 + g : g in [0,8)}.
"""

import numpy as np

import concourse.bacc as bacc
import concourse.mybir as mybir
import concourse.tile as tile
from concourse.bass_utils import run_bass_kernel_spmd

F32 = mybir.dt.float32
F16 = mybir.dt.float16

B, F, NB, NCLS = 65536, 6, 4, 10
CORES = 8
ROWS = B // CORES          # 8192 rows per core
G = 8                      # row-groups per supertile
ST_ROWS = 128 * G          # 1024 rows per supertile
N_ST = ROWS // ST_ROWS     # 8 supertiles
TEMP = 0.1

BLOCKS = [[0], [1, 2], [3, 4], [5, 6], [7]]

_NC_CACHE = {}


def _build_nc():
    nc = bacc.Bacc("TRN2", target_bir_lowering=False, debug=False)

    zt = nc.dram_tensor("zt", [128, N_ST, G, 24], F16, kind="ExternalInput")
    l3p = nc.dram_tensor("l3p", [128, 2, 160], F16, kind="ExternalInput")
    identf = nc.dram_tensor("identf", [128, 128], F32, kind="ExternalInput")
    outc = nc.dram_tensor("outc", [128, N_ST, G, NCLS], F32, kind="ExternalOutput")

    with tile.TileContext(nc) as tc:
        with (
            tc.tile_pool(name="big", bufs=1) as big,
            tc.tile_pool(name="work", bufs=3) as work,
            tc.tile_pool(name="io", bufs=2) as io,
            tc.tile_pool(name="wts", bufs=3) as wts,
            tc.tile_pool(name="ps_t", bufs=2, space="PSUM") as ps_t,
            tc.tile_pool(name="ps_m", bufs=2, space="PSUM") as ps_m,
        ):
            # singly-written resident tensors
            z_sb = big.tile([128, N_ST, G, 24], F16)
            sig = big.tile([128, N_ST, G, 24], F16)
            u_t = big.tile([128, N_ST, G, 16], F16)
            vp2 = big.tile([128, N_ST, G, 16, 2], F16)
            a_t = big.tile([128, N_ST, G, 16], F16)
            msb = big.tile([128, N_ST, G, 160], F16)
            l3_sb = big.tile([128, 2, 160], F16)
            id_sb = big.tile([128, 128], F32)

            # input DMAs: z for the first three supertiles, then consts
            # (needed ~2us later), then the rest of z
            nc.sync.dma_start(z_sb[:, 0:1], zt[:, 0:1])
            nc.sync.dma_start(z_sb[:, 1:3], zt[:, 1:3])
            nc.sync.dma_start(id_sb[:, :], identf[:, :])
            nc.sync.dma_start(l3_sb[:, :, :], l3p[:, :, :])
            nc.sync.dma_start(z_sb[:, 3:5], zt[:, 3:5])
            nc.sync.dma_start(z_sb[:, 5:7], zt[:, 5:7])
            nc.sync.dma_start(z_sb[:, 7:8], zt[:, 7:8])

            def blk_view(t, blk, tail):
                # [128, n*G] + tail view of a big tensor's block slice
                n = len(blk)
                return t[:, blk[0]:blk[0] + n].rearrange(
                    f"p n g {tail} -> p (n g) {tail}")

            def sigmoid(blk):
                k0, n = blk[0], len(blk)
                nc.scalar.activation(
                    sig[:, k0:k0 + n], z_sb[:, k0:k0 + n],
                    mybir.ActivationFunctionType.Sigmoid,
                )

            def krons(blk, eng):
                # u = s2 x s3;  vp2[.., vv, t] = s4[k4]*s5[k5] (dup via
                # stride-0 reads, no separate dup op)
                ng = len(blk) * G
                sg = blk_view(sig, blk, "c")            # [128, ng, 24]
                eng.tensor_mul(
                    blk_view(u_t, blk, "a").rearrange(
                        "p s (i j) -> p s i j", j=NB),
                    sg[:, :, 8:12].unsqueeze(3).broadcast_to((128, ng, NB, NB)),
                    sg[:, :, 12:16].unsqueeze(2).broadcast_to((128, ng, NB, NB)),
                )
                for t in range(2):
                    eng.tensor_mul(
                        blk_view(vp2, blk, "v t")[:, :, :, t].rearrange(
                            "p s (i j) -> p s i j", j=NB),
                        sg[:, :, 16:20].unsqueeze(3).broadcast_to(
                            (128, ng, NB, NB)),
                        sg[:, :, 20:24].unsqueeze(2).broadcast_to(
                            (128, ng, NB, NB)),
                    )

            def a_kron(blk, eng):
                ng = len(blk) * G
                sg = blk_view(sig, blk, "c")
                eng.tensor_mul(
                    blk_view(a_t, blk, "a").rearrange(
                        "p s (i j) -> p s i j", j=NB),
                    sg[:, :, 0:4].unsqueeze(3).broadcast_to((128, ng, NB, NB)),
                    sg[:, :, 4:8].unsqueeze(2).broadcast_to((128, ng, NB, NB)),
                )

            def stage_C(blk):
                # prod + pairwise-add tree + output DMA for a block
                k0, n = blk[0], len(blk)
                ng = n * G
                prodc = work.tile([128, ng, NCLS, 16], F16, tag="prodc")
                nc.vector.tensor_mul(
                    prodc[:, :, :, :],
                    blk_view(a_t, blk, "a").unsqueeze(2).broadcast_to(
                        (128, ng, NCLS, 16)),
                    blk_view(msb, blk, "ca").rearrange(
                        "p s (c a) -> p s c a", a=16),
                )
                f1 = work.tile([128, ng, NCLS, 8], F16, tag="f1")
                nc.vector.tensor_add(
                    f1[:, :, :, :], prodc[:, :, :, 0:8], prodc[:, :, :, 8:16])
                f2 = work.tile([128, ng, NCLS, 4], F16, tag="f2")
                nc.vector.tensor_add(
                    f2[:, :, :, :], f1[:, :, :, 0:4], f1[:, :, :, 4:8])
                f3 = work.tile([128, ng, NCLS, 2], F16, tag="f3")
                nc.vector.tensor_add(
                    f3[:, :, :, :], f2[:, :, :, 0:2], f2[:, :, :, 2:4])
                oq = io.tile([128, ng, NCLS], F32, tag="oq")
                nc.vector.tensor_add(oq[:, :, :], f3[:, :, :, 0], f3[:, :, :, 1])
                nc.sync.dma_start(
                    outc[:, k0:k0 + n],
                    oq[:, :, :].rearrange("p (n g) c -> p n g c", n=n))

            def stage_B(k):
                # transposes + matmuls + M evacuation for supertile k
                bm = state[k]
                tp = ps_t.tile([128, G, 128], F32, tag="tp")
                for q in range(G):
                    nc.tensor.transpose(
                        tp[:, q, :], bm[:, q, :].bitcast(F32), id_sb[:, :])
                bmt = wts.tile([128, G, 128], F32, tag="bmt")
                nc.scalar.copy(bmt[:, :, :], tp[:, :, :])
                for m in range(2):
                    mps = ps_m.tile([128, 4, 256], F32, tag="m")
                    for qq in range(4):
                        q = m * 4 + qq
                        w16 = bmt[:, q, :].bitcast(F16).rearrange(
                            "p (j t) -> p t j", t=2)
                        nc.tensor.matmul(
                            mps[:, qq, 0:160], w16[:, 0, :], l3_sb[:, 0, :],
                            start=True, stop=False,
                        )
                        nc.tensor.matmul(
                            mps[:, qq, 0:160], w16[:, 1, :], l3_sb[:, 1, :],
                            start=False, stop=True,
                        )
                    nc.scalar.copy(
                        msb[:, k, m * 4:(m + 1) * 4, :], mps[:, :, 0:160])

            state = {}

            # sigmoids for every block up front (scalar)
            for blk in BLOCKS:
                sigmoid(blk)
            # block 0 krons on the ramp-idle DVE; block 1 on gpsimd
            krons(BLOCKS[0], nc.vector)
            krons(BLOCKS[1], nc.gpsimd)

            GP_BM = ()          # gpsimd bm build measured ~4us/supertile: off

            for bi, blk in enumerate(BLOCKS):
                if bi + 2 < len(BLOCKS):
                    krons(BLOCKS[bi + 2], nc.gpsimd)
                a_kron(blk, nc.gpsimd)
                for k in blk:
                    # Bm[p,g, vv*16 + uu] = u[p,g,uu] * v[p,g,vv]
                    bm = work.tile([128, G, 256], F16, tag="bm")
                    if k in GP_BM:
                        nc.gpsimd.tensor_mul(
                            bm[:, :, :].rearrange("p g (i u) -> p g i u", u=16),
                            u_t[:, k].unsqueeze(2).broadcast_to(
                                (128, G, 16, 16)),
                            vp2[:, k, :, :, 0].unsqueeze(3).broadcast_to(
                                (128, G, 16, 16)),
                        )
                    else:
                        nc.vector.tensor_mul(
                            bm[:, :, :].rearrange(
                                "p g (i j t) -> p g i j t", j=8, t=2),
                            u_t[:, k].rearrange("p g (j t) -> p g j t", t=2)
                                .unsqueeze(2).broadcast_to((128, G, 16, 8, 2)),
                            vp2[:, k].unsqueeze(3).broadcast_to(
                                (128, G, 16, 8, 2)),
                        )
                    state[k] = bm
                    stage_B(k)
                if bi > 0:
                    stage_C(BLOCKS[bi - 1])
            stage_C(BLOCKS[-1])

    nc.compile()
    return nc


def _host_prep(x, beta, leaves2classes):
    x = np.asarray(x, dtype=np.float32)
    beta = np.asarray(beta, dtype=np.float32)
    L = np.asarray(leaves2classes, dtype=np.float32)

    w = np.linspace(1.0, float(NB), NB, dtype=np.float32)
    bs = np.sort(beta)
    b = np.concatenate([np.zeros(1, np.float32), np.cumsum(-bs, dtype=np.float32)])

    # z[i, f*4+k] = (x[i,f]*w[k] + b[k]) / T
    z = (x[:, :, None] * w[None, None, :] + b[None, None, :]) / np.float32(TEMP)
    z = z.reshape(B, F * NB).astype(np.float16)

    # L3r[b_leaf, a, c] = L[a*256 + b_leaf, c]
    L3r = L.reshape(16, 256, NCLS).transpose(1, 0, 2)      # [256, 16, 10]
    # device Bm column order: col = vv*16 + uu, packed pair (2J, 2J+1):
    #   J = vv*8 + uu//2, T = uu%2;  b_leaf = uu*16 + vv
    J = np.arange(128)
    l3p = np.empty((128, 2, 160), np.float32)
    for T in range(2):
        uu = 2 * (J % 8) + T
        vv = J // 8
        rows = L3r[uu * 16 + vv]                           # [128, 16, 10]
        # columns (c, a)-major: col = c*16 + a
        l3p[:, T, :] = rows.transpose(0, 2, 1).reshape(128, 160)
    l3p = l3p.astype(np.float16)

    ident = np.eye(128, dtype=np.float32)
    return z, l3p, ident


def _prep_in_maps(x, beta, leaves2classes):
    z, l3p, ident = _host_prep(x, beta, leaves2classes)
    in_maps = []
    for c in range(CORES):
        zc = z[c * ROWS:(c + 1) * ROWS].reshape(N_ST, 128, G, 24)
        zc = np.ascontiguousarray(zc.transpose(1, 0, 2, 3))
        in_maps.append({"zt": zc, "l3p": l3p, "identf": ident})
    return in_maps


def kernel(x, beta, leaves2classes):
    in_maps = _prep_in_maps(x, beta, leaves2classes)

    if "nc" not in _NC_CACHE:
        _NC_CACHE["nc"] = _build_nc()
    nc = _NC_CACHE["nc"]

    res = run_bass_kernel_spmd(nc, in_maps, core_ids=list(range(CORES)))
    outs = []
    for r in res.results:
        oc = r["outc"]                                     # [128, N_ST, G, 10]
        outs.append(oc.transpose(1, 0, 2, 3).reshape(ROWS, NCLS))
    return np.concatenate(outs, axis=0).astype(np.float32)


# revision 17
# speedup vs baseline: 1.2353x; 1.0212x over previous
"""DNDT forward kernel for Trainium2 (8 NeuronCores, data-parallel).

Math (matches the reference):
    w = [1,2,3,4];  b = [0, cumsum(-sort(beta))]
    sigma[i,f,k] = sigmoid((x[i,f]*w[k] + b[k]) / T)            [B, 6, 4]
    leaves[i]    = kron(sigma[i,0], ..., sigma[i,5])            [B, 4096]
    out          = leaves @ L                                   [B, 10]

Restructured to avoid materializing the 4096-wide leaves:
    A[i,a]  = kron(s0, s1)          a = k0*4+k1      in [0,16)
    Bm[i,b] = kron(s2, s3, s4, s5)  b in [0,256)
    M[i,(c,a)] = sum_b Bm[i,b] * L3[b, (c,a)]   (PE matmul, K=256 in 2 chunks)
    out[i,c]   = sum_a A[i,a] * M[i,(c,a)]      (DVE multiply + pairwise adds)

Device tricks:
  - z = (x*w+b)/T precomputed on host, shipped f16 in supertile layout.
  - Bm columns ordered so consecutive f16 pairs belong to the two K-chunks
    (uu-parity).  One PE transpose of the f32-bitcast [128,128] tile yields
    both chunks' weights; LDWEIGHTS reads them with a stride-2 f16 AP.
    The compensating L3 row permutation is done on host.
  - M is (c,a)-major (host permutes L3 columns) so the A-contraction is a
    3-free-dim fp16 tensor_tensor at DVE 2x (stride-1 innermost for both
    operands, no duplication), followed by pairwise adds.
  - Cheap stages (sigmoid, krons, A, prod, add-tree) are fused across
    supertile blocks [0],[1,2],[3,4],[5,6],[7] to amortize per-op engine
    overhead; bm + transpose + matmul stay per-supertile for pipelining.
  - Supertile 0's krons run on the (otherwise idle at ramp) DVE; later
    blocks' krons run on GpSimd one block ahead of use.
  - PSUM->SBUF evacuation of M is split: m-group 0 on Scalar, 1 on GpSimd.
  - prod/tree/output for block b are emitted after block b+1's matmuls
    (software pipelining), so the DVE never waits on the PE/Scalar chain.

Per-core: 8192 rows as 8 supertiles of 1024 rows; partition p holds rows
{st*1024 + p*8 + g : g in [0,8)}.
"""

import numpy as np

import concourse.bacc as bacc
import concourse.mybir as mybir
import concourse.tile as tile
from concourse.bass_utils import run_bass_kernel_spmd

F32 = mybir.dt.float32
F16 = mybir.dt.float16

B, F, NB, NCLS = 65536, 6, 4, 10
CORES = 8
ROWS = B // CORES          # 8192 rows per core
G = 8                      # row-groups per supertile
ST_ROWS = 128 * G          # 1024 rows per supertile
N_ST = ROWS // ST_ROWS     # 8 supertiles
TEMP = 0.1

BLOCKS = [[0], [1, 2], [3, 4], [5, 6], [7]]

_NC_CACHE = {}


def _build_nc():
    nc = bacc.Bacc("TRN2", target_bir_lowering=False, debug=False)

    zt = nc.dram_tensor("zt", [128, N_ST, G, 24], F16, kind="ExternalInput")
    l3p = nc.dram_tensor("l3p", [128, 2, 160], F16, kind="ExternalInput")
    identf = nc.dram_tensor("identf", [128, 128], F32, kind="ExternalInput")
    outc = nc.dram_tensor("outc", [128, N_ST, G, NCLS], F32, kind="ExternalOutput")

    with tile.TileContext(nc) as tc:
        with (
            tc.tile_pool(name="big", bufs=1) as big,
            tc.tile_pool(name="work", bufs=3) as work,
            tc.tile_pool(name="io", bufs=2) as io,
            tc.tile_pool(name="wts", bufs=3) as wts,
            tc.tile_pool(name="ps_t", bufs=2, space="PSUM") as ps_t,
            tc.tile_pool(name="ps_m", bufs=2, space="PSUM") as ps_m,
        ):
            # singly-written resident tensors
            z_sb = big.tile([128, N_ST, G, 24], F16)
            sig = big.tile([128, N_ST, G, 24], F16)
            u_t = big.tile([128, N_ST, G, 16], F16)
            vp2 = big.tile([128, N_ST, G, 16, 2], F16)
            a_t = big.tile([128, N_ST, G, 16], F16)
            msb = big.tile([128, N_ST, G, 160], F16)
            l3_sb = big.tile([128, 2, 160], F16)
            id_sb = big.tile([128, 128], F32)

            # input DMAs: z for the first three supertiles, then consts
            # (needed ~2us later), then the rest of z
            nc.sync.dma_start(z_sb[:, 0:1], zt[:, 0:1])
            nc.sync.dma_start(z_sb[:, 1:3], zt[:, 1:3])
            nc.sync.dma_start(id_sb[:, :], identf[:, :])
            nc.sync.dma_start(l3_sb[:, :, :], l3p[:, :, :])
            nc.sync.dma_start(z_sb[:, 3:5], zt[:, 3:5])
            nc.sync.dma_start(z_sb[:, 5:7], zt[:, 5:7])
            nc.sync.dma_start(z_sb[:, 7:8], zt[:, 7:8])

            def blk_view(t, blk, tail):
                # [128, n*G] + tail view of a big tensor's block slice
                n = len(blk)
                return t[:, blk[0]:blk[0] + n].rearrange(
                    f"p n g {tail} -> p (n g) {tail}")

            def sigmoid(blk):
                k0, n = blk[0], len(blk)
                nc.scalar.activation(
                    sig[:, k0:k0 + n], z_sb[:, k0:k0 + n],
                    mybir.ActivationFunctionType.Sigmoid,
                )

            def krons(blk, eng):
                # u = s2 x s3;  vp2[.., vv, t] = s4[k4]*s5[k5] (dup via
                # stride-0 reads, no separate dup op)
                ng = len(blk) * G
                sg = blk_view(sig, blk, "c")            # [128, ng, 24]
                eng.tensor_mul(
                    blk_view(u_t, blk, "a").rearrange(
                        "p s (i j) -> p s i j", j=NB),
                    sg[:, :, 8:12].unsqueeze(3).broadcast_to((128, ng, NB, NB)),
                    sg[:, :, 12:16].unsqueeze(2).broadcast_to((128, ng, NB, NB)),
                )
                for t in range(2):
                    eng.tensor_mul(
                        blk_view(vp2, blk, "v t")[:, :, :, t].rearrange(
                            "p s (i j) -> p s i j", j=NB),
                        sg[:, :, 16:20].unsqueeze(3).broadcast_to(
                            (128, ng, NB, NB)),
                        sg[:, :, 20:24].unsqueeze(2).broadcast_to(
                            (128, ng, NB, NB)),
                    )

            def a_kron(blk, eng):
                ng = len(blk) * G
                sg = blk_view(sig, blk, "c")
                eng.tensor_mul(
                    blk_view(a_t, blk, "a").rearrange(
                        "p s (i j) -> p s i j", j=NB),
                    sg[:, :, 0:4].unsqueeze(3).broadcast_to((128, ng, NB, NB)),
                    sg[:, :, 4:8].unsqueeze(2).broadcast_to((128, ng, NB, NB)),
                )

            def stage_C(blk):
                # prod + pairwise-add tree + output DMA for a block
                k0, n = blk[0], len(blk)
                ng = n * G
                prodc = work.tile([128, ng, NCLS, 16], F16, tag="prodc")
                nc.vector.tensor_mul(
                    prodc[:, :, :, :],
                    blk_view(a_t, blk, "a").unsqueeze(2).broadcast_to(
                        (128, ng, NCLS, 16)),
                    blk_view(msb, blk, "ca").rearrange(
                        "p s (c a) -> p s c a", a=16),
                )
                f1 = work.tile([128, ng, NCLS, 8], F16, tag="f1")
                nc.vector.tensor_add(
                    f1[:, :, :, :], prodc[:, :, :, 0:8], prodc[:, :, :, 8:16])
                f2 = work.tile([128, ng, NCLS, 4], F16, tag="f2")
                nc.vector.tensor_add(
                    f2[:, :, :, :], f1[:, :, :, 0:4], f1[:, :, :, 4:8])
                f3 = work.tile([128, ng, NCLS, 2], F16, tag="f3")
                nc.vector.tensor_add(
                    f3[:, :, :, :], f2[:, :, :, 0:2], f2[:, :, :, 2:4])
                oq = io.tile([128, ng, NCLS], F32, tag="oq")
                nc.vector.tensor_add(oq[:, :, :], f3[:, :, :, 0], f3[:, :, :, 1])
                nc.sync.dma_start(
                    outc[:, k0:k0 + n],
                    oq[:, :, :].rearrange("p (n g) c -> p n g c", n=n))

            def stage_B(k):
                # transposes + matmuls + M evacuation for supertile k
                bm = state[k]
                tp = ps_t.tile([128, G, 128], F32, tag="tp")
                for q in range(G):
                    nc.tensor.transpose(
                        tp[:, q, :], bm[:, q, :].bitcast(F32), id_sb[:, :])
                bmt = wts.tile([128, G, 128], F32, tag="bmt")
                nc.scalar.copy(bmt[:, :, :], tp[:, :, :])
                for m in range(2):
                    mps = ps_m.tile([128, 4, 256], F32, tag="m")
                    for qq in range(4):
                        q = m * 4 + qq
                        w16 = bmt[:, q, :].bitcast(F16).rearrange(
                            "p (j t) -> p t j", t=2)
                        nc.tensor.matmul(
                            mps[:, qq, 0:160], w16[:, 0, :], l3_sb[:, 0, :],
                            start=True, stop=False,
                        )
                        nc.tensor.matmul(
                            mps[:, qq, 0:160], w16[:, 1, :], l3_sb[:, 1, :],
                            start=False, stop=True,
                        )
                    nc.scalar.copy(
                        msb[:, k, m * 4:(m + 1) * 4, :], mps[:, :, 0:160])

            state = {}

            # PE warm-up: ~4us of zero matmuls with no input dependencies so
            # the HAM clock gate opens before real work arrives
            zs = big.tile([128, 128], F16)
            nc.gpsimd.memset(zs[:, :], 0)
            wtp = ps_t.tile([128, G, 128], F32, tag="tp")
            for w in range(40):
                nc.tensor.matmul(wtp[:, w % G, :], zs[:, :], zs[:, :],
                                 start=True, stop=True)

            # sigmoids for every block up front (scalar)
            for blk in BLOCKS:
                sigmoid(blk)
            # blocks 0-1 krons on the ramp-idle DVE; later blocks on gpsimd
            krons(BLOCKS[0], nc.vector)
            krons(BLOCKS[1], nc.vector)

            GP_BM = ()          # gpsimd bm build measured ~4us/supertile: off

            for bi, blk in enumerate(BLOCKS):
                if bi + 2 < len(BLOCKS):
                    krons(BLOCKS[bi + 2], nc.gpsimd)
                a_kron(blk, nc.gpsimd)
                for k in blk:
                    # Bm[p,g, vv*16 + uu] = u[p,g,uu] * v[p,g,vv]
                    bm = work.tile([128, G, 256], F16, tag="bm")
                    if k in GP_BM:
                        nc.gpsimd.tensor_mul(
                            bm[:, :, :].rearrange("p g (i u) -> p g i u", u=16),
                            u_t[:, k].unsqueeze(2).broadcast_to(
                                (128, G, 16, 16)),
                            vp2[:, k, :, :, 0].unsqueeze(3).broadcast_to(
                                (128, G, 16, 16)),
                        )
                    else:
                        nc.vector.tensor_mul(
                            bm[:, :, :].rearrange(
                                "p g (i j t) -> p g i j t", j=8, t=2),
                            u_t[:, k].rearrange("p g (j t) -> p g j t", t=2)
                                .unsqueeze(2).broadcast_to((128, G, 16, 8, 2)),
                            vp2[:, k].unsqueeze(3).broadcast_to(
                                (128, G, 16, 8, 2)),
                        )
                    state[k] = bm
                    stage_B(k)
                if bi > 0:
                    stage_C(BLOCKS[bi - 1])
            stage_C(BLOCKS[-1])

    nc.compile()
    return nc


def _host_prep(x, beta, leaves2classes):
    x = np.asarray(x, dtype=np.float32)
    beta = np.asarray(beta, dtype=np.float32)
    L = np.asarray(leaves2classes, dtype=np.float32)

    w = np.linspace(1.0, float(NB), NB, dtype=np.float32)
    bs = np.sort(beta)
    b = np.concatenate([np.zeros(1, np.float32), np.cumsum(-bs, dtype=np.float32)])

    # z[i, f*4+k] = (x[i,f]*w[k] + b[k]) / T
    z = (x[:, :, None] * w[None, None, :] + b[None, None, :]) / np.float32(TEMP)
    z = z.reshape(B, F * NB).astype(np.float16)

    # L3r[b_leaf, a, c] = L[a*256 + b_leaf, c]
    L3r = L.reshape(16, 256, NCLS).transpose(1, 0, 2)      # [256, 16, 10]
    # device Bm column order: col = vv*16 + uu, packed pair (2J, 2J+1):
    #   J = vv*8 + uu//2, T = uu%2;  b_leaf = uu*16 + vv
    J = np.arange(128)
    l3p = np.empty((128, 2, 160), np.float32)
    for T in range(2):
        uu = 2 * (J % 8) + T
        vv = J // 8
        rows = L3r[uu * 16 + vv]                           # [128, 16, 10]
        # columns (c, a)-major: col = c*16 + a
        l3p[:, T, :] = rows.transpose(0, 2, 1).reshape(128, 160)
    l3p = l3p.astype(np.float16)

    ident = np.eye(128, dtype=np.float32)
    return z, l3p, ident


def _prep_in_maps(x, beta, leaves2classes):
    z, l3p, ident = _host_prep(x, beta, leaves2classes)
    in_maps = []
    for c in range(CORES):
        zc = z[c * ROWS:(c + 1) * ROWS].reshape(N_ST, 128, G, 24)
        zc = np.ascontiguousarray(zc.transpose(1, 0, 2, 3))
        in_maps.append({"zt": zc, "l3p": l3p, "identf": ident})
    return in_maps


def kernel(x, beta, leaves2classes):
    in_maps = _prep_in_maps(x, beta, leaves2classes)

    if "nc" not in _NC_CACHE:
        _NC_CACHE["nc"] = _build_nc()
    nc = _NC_CACHE["nc"]

    res = run_bass_kernel_spmd(nc, in_maps, core_ids=list(range(CORES)))
    outs = []
    for r in res.results:
        oc = r["outc"]                                     # [128, N_ST, G, 10]
        outs.append(oc.transpose(1, 0, 2, 3).reshape(ROWS, NCLS))
    return np.concatenate(outs, axis=0).astype(np.float32)
